# revision 23
# baseline (speedup 1.0000x reference)
"""Trainium2 Bass kernel for CAM-style channel attention module.

Reference computation (per batch b):
    Q  = W @ X + bias          # 1x1 conv: [256,512]@[512,4096] -> [256,4096]
    E  = Q @ X^T / sqrt(4096)  # [256,512] channel-attention energy
    A  = softmax(E, axis=-1)
    out = gamma * (A @ X) + Q  # residual

Algebraic restructure (this version):
  1. E = (W G + b s^T)/64 with G = X X^T (Gram) and s = X @ 1.
     G is symmetric: only the upper-triangular 128-blocks are computed
     (1280 cols/n-tile instead of 2048); the 6 lower blocks are PE
     transposes of the upper ones. W G runs in fp32 (only 8+2 matmuls)
     so the dominant G diagonal (~4096) does not amplify W rounding.
  2. The host pre-transposes x into an n-partitioned bf16 copy, so the
     Gram contraction over n needs NO on-chip transposes at all (the
     old kernel spent 16k PE cycles/batch transposing X).
  3. Residual never materializes Q:  gamma*(A@X) + (W@X + b)
     = (W + gamma*A) @ X + b, a single fused bf16 matmul stage.
  4. softmax without max-subtraction (|E|/64 <= ~25, exp safe in fp32).

Device strategy: 8 NeuronCores, data-parallel over batch, 2 per core.
PE stream: G(b0) | G(b1) with b0's {lower-T, s-row, WG+bias+softmax,
AT} interleaved | F(b0) with b1's mid-stages interleaved | F(b1).
PSUM budget (8 banks): psG 4 + psE 1 + psT 1 + psF 2.
DMA queues: xt on gpsimd, xb on scalar, consts+output on sync.
"""

import numpy as np
import ml_dtypes

import concourse.bass as bass
import concourse.tile as tile
from concourse import bacc, mybir
from concourse.bass_utils import run_bass_kernel_spmd

P = 128
NB = 2         # batches per core (B=16 over 8 cores)
C = 512        # input channels
C1 = 256       # conv output channels
HW = 4096      # H*W
CT = C // P    # 4 c-tiles
NT = HW // P   # 32 n-tiles
QT = C1 // P   # 2 q-tiles
NCH = 8        # x DMA chunks per tensor (xt: 4 n-tiles each; xb: 512 cols)
F32 = mybir.dt.float32
F32R = mybir.dt.float32r
BF16 = mybir.dt.bfloat16
SCALE = 1.0 / 64.0  # 1/sqrt(HW)

N_CORES = 8
USE_BIAS = True


def build_nc():
    nc = bacc.Bacc("TRN2", target_bir_lowering=False, debug=False,
                   num_devices=N_CORES)

    # host-prepped inputs
    xt_d = nc.dram_tensor("xt", [NB, NCH, P, 4, C], BF16,
                          kind="ExternalInput").ap()   # x^T chunks [n-part, nt, c]
    xb_d = nc.dram_tensor("xb", [NB, NCH, P, CT, 512], BF16,
                          kind="ExternalInput").ap()   # x chunks [c-part, ct, n]
    wt_f = nc.dram_tensor("wt_f", [P, CT, C1], F32,
                          kind="ExternalInput").ap()   # W^T tiled, fp32
    b_row = nc.dram_tensor("b_row", [1, C1], BF16,
                           kind="ExternalInput").ap()  # bias as a row
    bq = nc.dram_tensor("bq", [P, QT], F32, kind="ExternalInput").ap()
    gam = nc.dram_tensor("gam", [P, 1], F32, kind="ExternalInput").ap()
    out = nc.dram_tensor("out", [NB, C1, HW], BF16,
                         kind="ExternalOutput").ap()

    ident_dram = nc.inline_tensor(np.eye(P, dtype=ml_dtypes.bfloat16),
                                  name="ident")

    # upper-tri block list (ci < cj) for the 6 transposed lower blocks
    LOWER = [(ci, cj) for ci in range(CT) for cj in range(ci + 1, CT)]

    with tile.TileContext(nc) as tc:
        with (
            tc.tile_pool(name="const", bufs=1) as const,
            tc.tile_pool(name="xt", bufs=2 * NCH) as xt_pool,
            tc.tile_pool(name="xb", bufs=2 * NCH) as xb_pool,
            tc.tile_pool(name="gsb", bufs=2) as gsb_pool,
            tc.tile_pool(name="gtmp", bufs=2) as gtmp_pool,
            tc.tile_pool(name="sm", bufs=2) as sm_pool,
            tc.tile_pool(name="srow", bufs=2) as srow_pool,
            tc.tile_pool(name="lhsf", bufs=4) as lhsf_pool,
            tc.tile_pool(name="osb", bufs=4) as osb_pool,
            tc.tile_pool(name="psG", bufs=4, space="PSUM") as psG,
            tc.tile_pool(name="psX", bufs=1, space="PSUM") as psX,
            tc.tile_pool(name="psF", bufs=3, space="PSUM") as psF,
        ):
            # ---- constants (sync queue) ----
            ident = const.tile([P, P], BF16)
            nc.sync.dma_start(out=ident, in_=ident_dram.ap())
            wtf_sb = const.tile([P, CT, C1], F32)
            nc.sync.dma_start(out=wtf_sb, in_=wt_f)
            wtfr_sb = const.tile([P, CT, C1], F32R)
            nc.gpsimd.dma_start(out=wtfr_sb, in_=wt_f)
            brow_sb = const.tile([1, C1], BF16)
            nc.sync.dma_start(out=brow_sb, in_=b_row)
            bq_sb = const.tile([P, QT], F32)
            nc.sync.dma_start(out=bq_sb, in_=bq)
            gam_sb = const.tile([P, 1], F32)
            nc.sync.dma_start(out=gam_sb, in_=gam)

            out_r = out.rearrange("b (t p) n -> b p t n", p=P)

            st = [dict() for _ in range(NB)]

            # ---- x DMAs spread over four queues (per-queue read BW is
            # only ~110-180 GB/s): early-needed xt(b0) alternates the two
            # fastest queues (scalar HWDGE / gpsimd SWDGE); xb odds go on
            # vector; sync carries consts, xt1-odds, then output.
            for bi in range(NB):
                st[bi]["xt"] = [None] * NCH
                st[bi]["xb"] = [None] * NCH

            def issue_xt(bi, j, eng):
                t = xt_pool.tile([P, 4, C], BF16, tag="xt",
                                 name=f"xt{bi}_{j}")
                eng.dma_start(out=t, in_=xt_d[bi, j])
                st[bi]["xt"][j] = t

            def issue_xb(bi, j, eng):
                t = xb_pool.tile([P, CT, 512], BF16, tag="xb",
                                 name=f"xb{bi}_{j}")
                eng.dma_start(out=t, in_=xb_d[bi, j])
                st[bi]["xb"][j] = t

            # Queue plan (rates: gpsimd ~2.6us/chunk, scalar ~3.7,
            # sync ~4.2, shared HBM): scalar takes exactly 8 early
            # chunks (one sem each -> no issue-waits blocking ScalarE
            # compute): xt0-odds + xb0-evens. gpsimd: xt0-evens,
            # xb0-odds, xt1. sync: consts, xb1 (early, it is free until
            # the output stream starts ~55us). Output groups alternate
            # sync/gpsimd so neither queue backlogs the final drain.
            for j in range(NCH):
                issue_xb(0, j, nc.scalar)
            for j in range(NCH):
                issue_xt(0, j, nc.gpsimd)
            for j in range(NCH):
                issue_xb(1, j, nc.sync)
            for j in range(NCH):
                issue_xt(1, j, nc.gpsimd)

            # ---- HAM warm-up: dummy matmuls while first x chunk lands.
            # Uses a memset tile, NOT the DMA'd identity, so the PE can
            # start ramping immediately instead of waiting for sync-queue
            # init (~8us).
            warm_in = const.tile([P, P], BF16)
            nc.vector.memset(warm_in, 0.25)
            ps_w = psF.tile([P, C], F32, tag="po", name="warm")
            NWARM = 100
            for wj in range(NWARM):
                nc.tensor.matmul(ps_w[:, :P], warm_in, warm_in,
                                 start=(wj == 0), stop=(wj == NWARM - 1))

            # ---------------- stage emitters ----------------
            def emit_G_nt(bi, nt):
                if nt == 0:
                    st[bi]["psg"] = [
                        psG.tile([P, C], F32, tag="g", name=f"g{bi}_{ci}")
                        for ci in range(CT)]
                ch = st[bi]["xt"][nt // 4]
                for ci in range(CT):
                    nc.tensor.matmul(
                        st[bi]["psg"][ci][:, ci * P:],
                        ch[:, nt % 4, ci * P:(ci + 1) * P],
                        ch[:, nt % 4, ci * P:],
                        start=(nt == 0), stop=(nt == NT - 1))

            def emit_s_reduce(bi):
                # s4[p, ct] = row-sums of x over n. Chunk-collapse with a
                # bf16 DVE add-chain (0.7us/add) as chunks land, then a
                # single free-dim reduce; 18x cheaper than reducing every
                # chunk separately.
                acc = sm_pool.tile([P, CT, 512], BF16, tag="sacc",
                                   name=f"sacc{bi}")
                nc.vector.tensor_add(out=acc, in0=st[bi]["xb"][0],
                                     in1=st[bi]["xb"][1])
                for j in range(2, NCH):
                    nc.vector.tensor_add(out=acc, in0=acc,
                                         in1=st[bi]["xb"][j])
                s4 = sm_pool.tile([P, CT], F32, tag="s4", name=f"s4{bi}")
                nc.vector.reduce_sum(s4, acc, axis=mybir.AxisListType.X)
                s4b = sm_pool.tile([P, CT], BF16, tag="s4b", name=f"s4b{bi}")
                nc.vector.tensor_copy(s4b, s4)
                st[bi]["s4b"] = s4b

            def emit_G_evac(bi):
                # alternate scalar/vector so the psG banks free fast
                gsb = gsb_pool.tile([P, CT, C], F32R, name=f"gsb{bi}")
                gtmp = gtmp_pool.tile([P, len(LOWER), P], BF16,
                                      name=f"gt{bi}")
                copies = []
                for ci in range(CT):
                    copies.append((gsb[:, ci, ci * P:],
                                   st[bi]["psg"][ci][:, ci * P:]))
                    for cj in range(ci + 1, CT):
                        k = LOWER.index((ci, cj))
                        copies.append((gtmp[:, k, :],
                                       st[bi]["psg"][ci][:, cj * P:(cj + 1) * P]))
                for i, (dst, src) in enumerate(copies):
                    if i % 2 == 0:
                        nc.scalar.copy(out=dst, in_=src)
                    else:
                        nc.vector.tensor_copy(dst, src)
                st[bi]["gsb"] = gsb
                st[bi]["gtmp"] = gtmp

            def emit_G_lower(bi):
                # lower blocks: Gsb[:, cj, ci-block] = upper(ci, cj)^T
                for k, (ci, cj) in enumerate(LOWER):
                    pst = psX.tile([P, C], F32, tag="x", name=f"glt{bi}")
                    nc.tensor.matmul(pst[:, :P], st[bi]["gtmp"][:, k, :],
                                     ident, start=True, stop=True)
                    dst = st[bi]["gsb"][:, cj, ci * P:(ci + 1) * P]
                    if k % 2 == 0:
                        nc.scalar.copy(out=dst, in_=pst[:, :P])
                    else:
                        nc.vector.tensor_copy(dst, pst[:, :P])

            def emit_srow(bi):
                # s4 [P, CT] -> s_row [1, C]: transpose one column at a
                # time (out [1, 128] lands on partition 0 — engines may
                # not read PSUM starting at partition > 0)
                pst = psX.tile([P, C], F32, tag="x", name=f"s4t{bi}")
                for ci in range(CT):
                    nc.tensor.matmul(pst[:1, ci * P:(ci + 1) * P],
                                     st[bi]["s4b"][:, ci:ci + 1], ident,
                                     start=True, stop=True)
                srow = srow_pool.tile([1, C], BF16, name=f"srow{bi}")
                nc.scalar.copy(out=srow, in_=pst[:1, :])
                st[bi]["srow"] = srow

            def emit_WGq(bi, qi):
                # E[qi] = W G + b (x) s  (fp32 + K=1 matmuls), then
                # softmax straight off the PSUM bank (psE bufs=1: the
                # whole per-qi pipeline completes before the next qi).
                if qi == 0:
                    st[bi]["a"] = sm_pool.tile([P, QT, C], BF16, tag="a",
                                               name=f"a{bi}")
                pse = psX.tile([P, C], F32, tag="x", name=f"e{bi}{qi}")
                for ct in range(CT):
                    nc.tensor.matmul(
                        pse, wtfr_sb[:, ct, qi * P:(qi + 1) * P],
                        st[bi]["gsb"][:, ct, :],
                        start=(ct == 0),
                        stop=(not USE_BIAS and ct == CT - 1))
                if USE_BIAS:
                    nc.tensor.matmul(
                        pse, brow_sb[:, qi * P:(qi + 1) * P],
                        st[bi]["srow"], start=False, stop=True)
                a_f = sm_pool.tile([P, C], BF16, tag="af")
                rs = sm_pool.tile([P, 1], F32, tag="rs")
                nc.scalar.activation(
                    out=a_f, in_=pse,
                    func=mybir.ActivationFunctionType.Exp,
                    scale=SCALE, accum_out=rs)
                rc = sm_pool.tile([P, 1], F32, tag="rc")
                nc.vector.reciprocal(rc, rs)
                sc = sm_pool.tile([P, 1], F32, tag="sc")
                nc.vector.tensor_mul(sc, rc, gam_sb)
                nc.vector.tensor_scalar_mul(st[bi]["a"][:, qi, :], a_f, sc)

            def emit_ATcombine(bi, qi):
                # transpose a_scaled[qi] and fold W^T in; separate tile
                # per qi so F(qi=0) need not wait for the qi=1 half
                lhsf = lhsf_pool.tile([P, CT, P], BF16, tag="lhsf",
                                      name=f"lhsf{bi}_{qi}")
                st[bi][f"lhsf{qi}"] = lhsf
                a_scaled = st[bi]["a"]
                ps_at = psX.tile([P, C], F32, tag="x", name=f"at{bi}")
                for ct in range(CT):
                    nc.tensor.matmul(
                        ps_at[:, ct * P:(ct + 1) * P],
                        a_scaled[:, qi, ct * P:(ct + 1) * P], ident,
                        start=True, stop=True)
                for ct in range(CT):
                    nc.vector.tensor_add(
                        out=lhsf[:, ct, :],
                        in0=ps_at[:, ct * P:(ct + 1) * P],
                        in1=wtf_sb[:, ct, qi * P:(qi + 1) * P])

            def emit_F_chunk(bi, qi, nch):
                lhsf = st[bi][f"lhsf{qi}"]
                half = nch % 2
                if half == 0:
                    st[bi]["osb"] = osb_pool.tile([P, 2 * C], BF16, tag="o",
                                                  name=f"osb{bi}")
                o_sb = st[bi]["osb"]
                ps_o = psF.tile([P, C], F32, tag="po", name="ps_o")
                rhs = st[bi]["xb"][nch]
                for ct in range(CT):
                    nc.tensor.matmul(ps_o, lhsf[:, ct, :], rhs[:, ct, :],
                                     start=(ct == 0), stop=(ct == CT - 1))
                oslice = o_sb[:, half * C:(half + 1) * C]
                if nch % 4 < 2:
                    nc.scalar.add(out=oslice, in_=ps_o,
                                  add=bq_sb[:, qi:qi + 1])
                else:
                    nc.vector.tensor_scalar_add(oslice, ps_o,
                                                bq_sb[:, qi:qi + 1])
                tail = bi == 1 and qi == QT - 1 and nch >= NCH - 2
                if tail:
                    nc.gpsimd.dma_start(
                        out=out_r[bi, :, qi, nch * C:(nch + 1) * C],
                        in_=o_sb[:, half * C:(half + 1) * C])
                elif half == 1:
                    eng = nc.sync if (nch // 2) % 2 == 0 else nc.gpsimd
                    eng.dma_start(
                        out=out_r[bi, :, qi, (nch - 1) * C:(nch + 1) * C],
                        in_=o_sb)

            # ---------------- the schedule ----------------
            for nt in range(NT):
                emit_G_nt(0, nt)
            if USE_BIAS:
                emit_s_reduce(0)        # DVE; fires as xb(b0) chunks land
            emit_G_evac(0)

            for nt in range(NT):
                emit_G_nt(1, nt)
                if nt == 2:
                    emit_G_lower(0)
                if USE_BIAS and nt == 20:
                    emit_srow(0)
                if nt == (22 if USE_BIAS else 6):
                    emit_WGq(0, 0)
                if nt == (24 if USE_BIAS else 8):
                    emit_ATcombine(0, 0)
                if nt == (26 if USE_BIAS else 10):
                    emit_WGq(0, 1)
                if nt == (28 if USE_BIAS else 12):
                    emit_ATcombine(0, 1)
            if USE_BIAS:
                emit_s_reduce(1)
            emit_G_evac(1)

            fseq = [(qi, nch) for qi in range(QT) for nch in range(NCH)]
            for g, (qi, nch) in enumerate(fseq):
                emit_F_chunk(0, qi, nch)
                if g == 1:
                    emit_G_lower(1)
                if USE_BIAS and g == 10:
                    emit_srow(1)
                if g == (11 if USE_BIAS else 3):
                    emit_WGq(1, 0)
                if g == (12 if USE_BIAS else 5):
                    emit_ATcombine(1, 0)
                if g == (13 if USE_BIAS else 7):
                    emit_WGq(1, 1)
                if g == (14 if USE_BIAS else 9):
                    emit_ATcombine(1, 1)
            for qi, nch in fseq:
                emit_F_chunk(1, qi, nch)
    nc.compile()
    return nc


_NC_CACHE = None


def _get_nc():
    global _NC_CACHE
    if _NC_CACHE is None:
        _NC_CACHE = build_nc()
    return _NC_CACHE


def make_in_maps(x, conv_w, conv_b, gamma):
    B = x.shape[0]
    x = np.asarray(x, dtype=np.float32)
    # c-partitioned bf16, chunk-major: [B, NCH, P, CT, 512]
    xb_full = np.ascontiguousarray(
        x.reshape(B, CT, P, NCH, 512).transpose(0, 3, 2, 1, 4)).astype(
            ml_dtypes.bfloat16)
    # n-partitioned bf16, chunk-major: [B, NCH, P, 4, C]
    xt_full = np.ascontiguousarray(
        x.reshape(B, C, NCH, 4, P).transpose(0, 2, 4, 3, 1)).astype(
            ml_dtypes.bfloat16)
    wm = conv_w.reshape(C1, C).astype(np.float32)
    wt_tiled = np.ascontiguousarray(
        wm.T.reshape(CT, P, C1).transpose(1, 0, 2))      # [P, CT, C1]
    b_np = conv_b.astype(np.float32)
    b_row = np.ascontiguousarray(b_np.reshape(1, C1)).astype(
        ml_dtypes.bfloat16)
    bq = np.ascontiguousarray(b_np.reshape(QT, P).T)     # [P, QT]
    gam = np.ascontiguousarray(
        np.broadcast_to(gamma.astype(np.float32).reshape(1, 1), (P, 1)))
    in_maps = []
    for ci in range(N_CORES):
        in_maps.append({
            "xt": np.ascontiguousarray(xt_full[NB * ci:NB * (ci + 1)]),
            "xb": np.ascontiguousarray(xb_full[NB * ci:NB * (ci + 1)]),
            "wt_f": wt_tiled,
            "b_row": b_row,
            "bq": bq,
            "gam": gam,
        })
    return in_maps


def kernel(x, conv_w, conv_b, gamma, trace=False):
    """Full inputs in, full output out. Shards batch over 8 NeuronCores."""
    nc = _get_nc()
    in_maps = make_in_maps(x, conv_w, conv_b, gamma)
    res = run_bass_kernel_spmd(nc, in_maps, core_ids=list(range(N_CORES)),
                               trace=trace)
    outs = [np.asarray(r["out"]).astype(np.float32).reshape(NB, C1, 64, 64)
            for r in res.results]
    full = np.concatenate(outs, axis=0)
    if trace:
        kernel.last_results = res
    return full


kernel.last_results = None


# revision 24
# speedup vs baseline: 1.0591x; 1.0591x over previous
"""Trainium2 Bass kernel for CAM-style channel attention module.

Reference computation (per batch b):
    Q  = W @ X + bias          # 1x1 conv: [256,512]@[512,4096] -> [256,4096]
    E  = Q @ X^T / sqrt(4096)  # [256,512] channel-attention energy
    A  = softmax(E, axis=-1)
    out = gamma * (A @ X) + Q  # residual

Algebraic restructure (this version):
  1. E = (W G + b s^T)/64 with G = X X^T (Gram) and s = X @ 1.
     G is symmetric: only the upper-triangular 128-blocks are computed
     (1280 cols/n-tile instead of 2048); the 6 lower blocks are PE
     transposes of the upper ones. W G runs in fp32 (only 8+2 matmuls)
     so the dominant G diagonal (~4096) does not amplify W rounding.
  2. The host pre-transposes x into an n-partitioned bf16 copy, so the
     Gram contraction over n needs NO on-chip transposes at all (the
     old kernel spent 16k PE cycles/batch transposing X).
  3. Residual never materializes Q:  gamma*(A@X) + (W@X + b)
     = (W + gamma*A) @ X + b, a single fused bf16 matmul stage.
  4. softmax without max-subtraction (|E|/64 <= ~25, exp safe in fp32).

Device strategy: 8 NeuronCores, data-parallel over batch, 2 per core.
PE stream: G(b0) | G(b1) with b0's {lower-T, s-row, WG+bias+softmax,
AT} interleaved | F(b0) with b1's mid-stages interleaved | F(b1).
PSUM budget (8 banks): psG 4 + psE 1 + psT 1 + psF 2.
DMA queues: xt on gpsimd, xb on scalar, consts+output on sync.
"""

import numpy as np
import ml_dtypes

import concourse.bass as bass
import concourse.tile as tile
from concourse import bacc, mybir
from concourse.bass_utils import run_bass_kernel_spmd

P = 128
NB = 2         # batches per core (B=16 over 8 cores)
C = 512        # input channels
C1 = 256       # conv output channels
HW = 4096      # H*W
CT = C // P    # 4 c-tiles
NT = HW // P   # 32 n-tiles
QT = C1 // P   # 2 q-tiles
NCH = 8        # x DMA chunks per tensor (xt: 4 n-tiles each; xb: 512 cols)
F32 = mybir.dt.float32
F32R = mybir.dt.float32r
BF16 = mybir.dt.bfloat16
SCALE = 1.0 / 64.0  # 1/sqrt(HW)

N_CORES = 8
USE_BIAS = True


def build_nc():
    nc = bacc.Bacc("TRN2", target_bir_lowering=False, debug=False,
                   num_devices=N_CORES)

    # host-prepped inputs
    xt_d = nc.dram_tensor("xt", [NB, NCH, P, 4, C], BF16,
                          kind="ExternalInput").ap()   # x^T chunks [n-part, nt, c]
    xb_d = nc.dram_tensor("xb", [NB, NCH, P, CT, 512], BF16,
                          kind="ExternalInput").ap()   # x chunks [c-part, ct, n]
    wt_f = nc.dram_tensor("wt_f", [P, CT, C1], F32,
                          kind="ExternalInput").ap()   # W^T tiled, fp32
    b_row = nc.dram_tensor("b_row", [1, C1], BF16,
                           kind="ExternalInput").ap()  # bias as a row
    bq = nc.dram_tensor("bq", [P, QT], F32, kind="ExternalInput").ap()
    gam = nc.dram_tensor("gam", [P, 1], F32, kind="ExternalInput").ap()
    out = nc.dram_tensor("out", [NB, C1, HW], BF16,
                         kind="ExternalOutput").ap()

    ident_dram = nc.inline_tensor(np.eye(P, dtype=ml_dtypes.bfloat16),
                                  name="ident")

    # upper-tri block list (ci < cj) for the 6 transposed lower blocks
    LOWER = [(ci, cj) for ci in range(CT) for cj in range(ci + 1, CT)]

    with tile.TileContext(nc) as tc:
        with (
            tc.tile_pool(name="const", bufs=1) as const,
            tc.tile_pool(name="xt", bufs=2 * NCH) as xt_pool,
            tc.tile_pool(name="xb", bufs=2 * NCH) as xb_pool,
            tc.tile_pool(name="gsb", bufs=2) as gsb_pool,
            tc.tile_pool(name="gtmp", bufs=2) as gtmp_pool,
            tc.tile_pool(name="sm", bufs=2) as sm_pool,
            tc.tile_pool(name="srow", bufs=2) as srow_pool,
            tc.tile_pool(name="lhsf", bufs=4) as lhsf_pool,
            tc.tile_pool(name="osb", bufs=4) as osb_pool,
            tc.tile_pool(name="psG", bufs=4, space="PSUM") as psG,
            tc.tile_pool(name="psX", bufs=1, space="PSUM") as psX,
            tc.tile_pool(name="psF", bufs=3, space="PSUM") as psF,
        ):
            # ---- constants (sync queue) ----
            ident = const.tile([P, P], BF16)
            nc.sync.dma_start(out=ident, in_=ident_dram.ap())
            wtf_sb = const.tile([P, CT, C1], F32)
            nc.sync.dma_start(out=wtf_sb, in_=wt_f)
            wtfr_sb = const.tile([P, CT, C1], F32R)
            nc.gpsimd.dma_start(out=wtfr_sb, in_=wt_f)
            brow_sb = const.tile([1, C1], BF16)
            nc.sync.dma_start(out=brow_sb, in_=b_row)
            bq_sb = const.tile([P, QT], F32)
            nc.sync.dma_start(out=bq_sb, in_=bq)
            gam_sb = const.tile([P, 1], F32)
            nc.sync.dma_start(out=gam_sb, in_=gam)

            out_r = out.rearrange("b (t p) n -> b p t n", p=P)

            st = [dict() for _ in range(NB)]

            # ---- x DMAs spread over four queues (per-queue read BW is
            # only ~110-180 GB/s): early-needed xt(b0) alternates the two
            # fastest queues (scalar HWDGE / gpsimd SWDGE); xb odds go on
            # vector; sync carries consts, xt1-odds, then output.
            for bi in range(NB):
                st[bi]["xt"] = [None] * NCH
                st[bi]["xb"] = [None] * NCH

            def issue_xt(bi, j, eng):
                t = xt_pool.tile([P, 4, C], BF16, tag="xt",
                                 name=f"xt{bi}_{j}")
                eng.dma_start(out=t, in_=xt_d[bi, j])
                st[bi]["xt"][j] = t

            def issue_xb(bi, j, eng):
                t = xb_pool.tile([P, CT, 512], BF16, tag="xb",
                                 name=f"xb{bi}_{j}")
                eng.dma_start(out=t, in_=xb_d[bi, j])
                st[bi]["xb"][j] = t

            # Queue plan (rates: gpsimd ~2.6us/chunk, scalar ~3.7,
            # sync ~4.2, shared HBM): scalar takes exactly 8 early
            # chunks (one sem each -> no issue-waits blocking ScalarE
            # compute): xt0-odds + xb0-evens. gpsimd: xt0-evens,
            # xb0-odds, xt1. sync: consts, xb1 (early, it is free until
            # the output stream starts ~55us). Output groups alternate
            # sync/gpsimd so neither queue backlogs the final drain.
            for j in range(1, NCH, 2):
                issue_xt(0, j, nc.scalar)
            for j in range(0, NCH, 2):
                issue_xb(0, j, nc.scalar)
            for j in range(0, NCH, 2):
                issue_xt(0, j, nc.gpsimd)
            for j in range(1, NCH, 2):
                issue_xb(0, j, nc.gpsimd)
            for j in range(NCH):
                issue_xb(1, j, nc.sync)
            for j in range(NCH):
                issue_xt(1, j, nc.gpsimd)

            # ---- HAM warm-up: dummy matmuls while first x chunk lands.
            # Uses a memset tile, NOT the DMA'd identity, so the PE can
            # start ramping immediately instead of waiting for sync-queue
            # init (~8us).
            warm_in = const.tile([P, P], BF16)
            nc.vector.memset(warm_in, 0.25)
            ps_w = psF.tile([P, C], F32, tag="po", name="warm")
            NWARM = 100
            for wj in range(NWARM):
                nc.tensor.matmul(ps_w[:, :P], warm_in, warm_in,
                                 start=(wj == 0), stop=(wj == NWARM - 1))

            # ---------------- stage emitters ----------------
            def emit_G_nt(bi, nt):
                if nt == 0:
                    st[bi]["psg"] = [
                        psG.tile([P, C], F32, tag="g", name=f"g{bi}_{ci}")
                        for ci in range(CT)]
                ch = st[bi]["xt"][nt // 4]
                for ci in range(CT):
                    nc.tensor.matmul(
                        st[bi]["psg"][ci][:, ci * P:],
                        ch[:, nt % 4, ci * P:(ci + 1) * P],
                        ch[:, nt % 4, ci * P:],
                        start=(nt == 0), stop=(nt == NT - 1))

            def emit_s_reduce(bi):
                # s4[p, ct] = row-sums of x over n. Chunk-collapse with a
                # bf16 DVE add-chain (0.7us/add) as chunks land, then a
                # single free-dim reduce; 18x cheaper than reducing every
                # chunk separately.
                acc = sm_pool.tile([P, CT, 512], BF16, tag="sacc",
                                   name=f"sacc{bi}")
                nc.vector.tensor_add(out=acc, in0=st[bi]["xb"][0],
                                     in1=st[bi]["xb"][1])
                for j in range(2, NCH):
                    nc.vector.tensor_add(out=acc, in0=acc,
                                         in1=st[bi]["xb"][j])
                s4 = sm_pool.tile([P, CT], F32, tag="s4", name=f"s4{bi}")
                nc.vector.reduce_sum(s4, acc, axis=mybir.AxisListType.X)
                s4b = sm_pool.tile([P, CT], BF16, tag="s4b", name=f"s4b{bi}")
                nc.vector.tensor_copy(s4b, s4)
                st[bi]["s4b"] = s4b

            def emit_G_evac(bi):
                # alternate scalar/vector so the psG banks free fast
                gsb = gsb_pool.tile([P, CT, C], F32R, name=f"gsb{bi}")
                gtmp = gtmp_pool.tile([P, len(LOWER), P], BF16,
                                      name=f"gt{bi}")
                copies = []
                for ci in range(CT):
                    copies.append((gsb[:, ci, ci * P:],
                                   st[bi]["psg"][ci][:, ci * P:]))
                    for cj in range(ci + 1, CT):
                        k = LOWER.index((ci, cj))
                        copies.append((gtmp[:, k, :],
                                       st[bi]["psg"][ci][:, cj * P:(cj + 1) * P]))
                for i, (dst, src) in enumerate(copies):
                    if i % 2 == 0:
                        nc.scalar.copy(out=dst, in_=src)
                    else:
                        nc.vector.tensor_copy(dst, src)
                st[bi]["gsb"] = gsb
                st[bi]["gtmp"] = gtmp

            def emit_G_lower(bi):
                # lower blocks: Gsb[:, cj, ci-block] = upper(ci, cj)^T
                for k, (ci, cj) in enumerate(LOWER):
                    pst = psX.tile([P, C], F32, tag="x", name=f"glt{bi}")
                    nc.tensor.matmul(pst[:, :P], st[bi]["gtmp"][:, k, :],
                                     ident, start=True, stop=True)
                    dst = st[bi]["gsb"][:, cj, ci * P:(ci + 1) * P]
                    if k % 2 == 0:
                        nc.scalar.copy(out=dst, in_=pst[:, :P])
                    else:
                        nc.vector.tensor_copy(dst, pst[:, :P])

            def emit_srow(bi):
                # s4 [P, CT] -> s_row [1, C]: transpose one column at a
                # time (out [1, 128] lands on partition 0 — engines may
                # not read PSUM starting at partition > 0)
                pst = psX.tile([P, C], F32, tag="x", name=f"s4t{bi}")
                for ci in range(CT):
                    nc.tensor.matmul(pst[:1, ci * P:(ci + 1) * P],
                                     st[bi]["s4b"][:, ci:ci + 1], ident,
                                     start=True, stop=True)
                srow = srow_pool.tile([1, C], BF16, name=f"srow{bi}")
                nc.scalar.copy(out=srow, in_=pst[:1, :])
                st[bi]["srow"] = srow

            def emit_WGq(bi, qi):
                # E[qi] = W G + b (x) s  (fp32 + K=1 matmuls), then
                # softmax straight off the PSUM bank (psE bufs=1: the
                # whole per-qi pipeline completes before the next qi).
                if qi == 0:
                    st[bi]["a"] = sm_pool.tile([P, QT, C], BF16, tag="a",
                                               name=f"a{bi}")
                pse = psX.tile([P, C], F32, tag="x", name=f"e{bi}{qi}")
                for ct in range(CT):
                    nc.tensor.matmul(
                        pse, wtfr_sb[:, ct, qi * P:(qi + 1) * P],
                        st[bi]["gsb"][:, ct, :],
                        start=(ct == 0),
                        stop=(not USE_BIAS and ct == CT - 1))
                if USE_BIAS:
                    nc.tensor.matmul(
                        pse, brow_sb[:, qi * P:(qi + 1) * P],
                        st[bi]["srow"], start=False, stop=True)
                a_f = sm_pool.tile([P, C], BF16, tag="af")
                rs = sm_pool.tile([P, 1], F32, tag="rs")
                nc.scalar.activation(
                    out=a_f, in_=pse,
                    func=mybir.ActivationFunctionType.Exp,
                    scale=SCALE, accum_out=rs)
                rc = sm_pool.tile([P, 1], F32, tag="rc")
                nc.vector.reciprocal(rc, rs)
                sc = sm_pool.tile([P, 1], F32, tag="sc")
                nc.vector.tensor_mul(sc, rc, gam_sb)
                nc.vector.tensor_scalar_mul(st[bi]["a"][:, qi, :], a_f, sc)

            def emit_ATcombine(bi, qi):
                # transpose a_scaled[qi] and fold W^T in; separate tile
                # per qi so F(qi=0) need not wait for the qi=1 half
                lhsf = lhsf_pool.tile([P, CT, P], BF16, tag="lhsf",
                                      name=f"lhsf{bi}_{qi}")
                st[bi][f"lhsf{qi}"] = lhsf
                a_scaled = st[bi]["a"]
                ps_at = psX.tile([P, C], F32, tag="x", name=f"at{bi}")
                for ct in range(CT):
                    nc.tensor.matmul(
                        ps_at[:, ct * P:(ct + 1) * P],
                        a_scaled[:, qi, ct * P:(ct + 1) * P], ident,
                        start=True, stop=True)
                for ct in range(CT):
                    nc.vector.tensor_add(
                        out=lhsf[:, ct, :],
                        in0=ps_at[:, ct * P:(ct + 1) * P],
                        in1=wtf_sb[:, ct, qi * P:(qi + 1) * P])

            def emit_F_chunk(bi, qi, nch):
                lhsf = st[bi][f"lhsf{qi}"]
                half = nch % 2
                if half == 0:
                    st[bi]["osb"] = osb_pool.tile([P, 2 * C], BF16, tag="o",
                                                  name=f"osb{bi}")
                o_sb = st[bi]["osb"]
                ps_o = psF.tile([P, C], F32, tag="po", name="ps_o")
                rhs = st[bi]["xb"][nch]
                for ct in range(CT):
                    nc.tensor.matmul(ps_o, lhsf[:, ct, :], rhs[:, ct, :],
                                     start=(ct == 0), stop=(ct == CT - 1))
                oslice = o_sb[:, half * C:(half + 1) * C]
                if nch % 4 < 2:
                    nc.scalar.add(out=oslice, in_=ps_o,
                                  add=bq_sb[:, qi:qi + 1])
                else:
                    nc.vector.tensor_scalar_add(oslice, ps_o,
                                                bq_sb[:, qi:qi + 1])
                tail = bi == 1 and qi == QT - 1 and nch >= NCH - 2
                if tail:
                    nc.gpsimd.dma_start(
                        out=out_r[bi, :, qi, nch * C:(nch + 1) * C],
                        in_=o_sb[:, half * C:(half + 1) * C])
                elif half == 1:
                    eng = nc.sync if (nch // 2) % 2 == 0 else nc.gpsimd
                    eng.dma_start(
                        out=out_r[bi, :, qi, (nch - 1) * C:(nch + 1) * C],
                        in_=o_sb)

            # ---------------- the schedule ----------------
            for nt in range(NT):
                emit_G_nt(0, nt)
            if USE_BIAS:
                emit_s_reduce(0)        # DVE; fires as xb(b0) chunks land
            emit_G_evac(0)

            for nt in range(NT):
                emit_G_nt(1, nt)
                if nt == 2:
                    emit_G_lower(0)
                if USE_BIAS and nt == 20:
                    emit_srow(0)
                if nt == (22 if USE_BIAS else 6):
                    emit_WGq(0, 0)
                if nt == (24 if USE_BIAS else 8):
                    emit_ATcombine(0, 0)
                if nt == (26 if USE_BIAS else 10):
                    emit_WGq(0, 1)
                if nt == (28 if USE_BIAS else 12):
                    emit_ATcombine(0, 1)
            if USE_BIAS:
                emit_s_reduce(1)
            emit_G_evac(1)

            fseq = [(qi, nch) for qi in range(QT) for nch in range(NCH)]
            for g, (qi, nch) in enumerate(fseq):
                emit_F_chunk(0, qi, nch)
                if g == 1:
                    emit_G_lower(1)
                if USE_BIAS and g == 10:
                    emit_srow(1)
                if g == (11 if USE_BIAS else 3):
                    emit_WGq(1, 0)
                if g == (12 if USE_BIAS else 5):
                    emit_ATcombine(1, 0)
                if g == (13 if USE_BIAS else 7):
                    emit_WGq(1, 1)
                if g == (14 if USE_BIAS else 9):
                    emit_ATcombine(1, 1)
            for qi, nch in fseq:
                emit_F_chunk(1, qi, nch)
    nc.compile()
    return nc


_NC_CACHE = None


def _get_nc():
    global _NC_CACHE
    if _NC_CACHE is None:
        _NC_CACHE = build_nc()
    return _NC_CACHE


def make_in_maps(x, conv_w, conv_b, gamma):
    B = x.shape[0]
    x = np.asarray(x, dtype=np.float32)
    # c-partitioned bf16, chunk-major: [B, NCH, P, CT, 512]
    xb_full = np.ascontiguousarray(
        x.reshape(B, CT, P, NCH, 512).transpose(0, 3, 2, 1, 4)).astype(
            ml_dtypes.bfloat16)
    # n-partitioned bf16, chunk-major: [B, NCH, P, 4, C]
    xt_full = np.ascontiguousarray(
        x.reshape(B, C, NCH, 4, P).transpose(0, 2, 4, 3, 1)).astype(
            ml_dtypes.bfloat16)
    wm = conv_w.reshape(C1, C).astype(np.float32)
    wt_tiled = np.ascontiguousarray(
        wm.T.reshape(CT, P, C1).transpose(1, 0, 2))      # [P, CT, C1]
    b_np = conv_b.astype(np.float32)
    b_row = np.ascontiguousarray(b_np.reshape(1, C1)).astype(
        ml_dtypes.bfloat16)
    bq = np.ascontiguousarray(b_np.reshape(QT, P).T)     # [P, QT]
    gam = np.ascontiguousarray(
        np.broadcast_to(gamma.astype(np.float32).reshape(1, 1), (P, 1)))
    in_maps = []
    for ci in range(N_CORES):
        in_maps.append({
            "xt": np.ascontiguousarray(xt_full[NB * ci:NB * (ci + 1)]),
            "xb": np.ascontiguousarray(xb_full[NB * ci:NB * (ci + 1)]),
            "wt_f": wt_tiled,
            "b_row": b_row,
            "bq": bq,
            "gam": gam,
        })
    return in_maps


def kernel(x, conv_w, conv_b, gamma, trace=False):
    """Full inputs in, full output out. Shards batch over 8 NeuronCores."""
    nc = _get_nc()
    in_maps = make_in_maps(x, conv_w, conv_b, gamma)
    res = run_bass_kernel_spmd(nc, in_maps, core_ids=list(range(N_CORES)),
                               trace=trace)
    outs = [np.asarray(r["out"]).astype(np.float32).reshape(NB, C1, 64, 64)
            for r in res.results]
    full = np.concatenate(outs, axis=0)
    if trace:
        kernel.last_results = res
    return full


kernel.last_results = None
